# revision 9
# baseline (speedup 1.0000x reference)
"""Distributed multi-head attention for TRN2 (8 NeuronCores).

Problem: b=2, t=2048, d=1024, h=16 heads, head_dim=64.
  out = softmax((q Wq^T)(k Wk^T)^T / 8) (v Wv^T) Wo^T + bo   (per head)

Sharding: core c -> batch i_b = c//4, head group i_h = c%4 (4 heads = 256
features). Each core projects Q/K/V for its batch+heads, runs attention,
then an 8-core AllToAll reshards head-major -> time-major so each core
computes the final projection for its 512-row time slice.

Device layouts (chosen so no on-chip transposes are needed):
  - activations streamed as X^T [d, t]; Q/K kept transposed [hn, t]
  - scores computed directly as S^T [t_k, t_q]; softmax denominator via an
    extra ones-column in the V operand of the P@V matmul
  - output projection computes Y^T [d, t_slice]; host transposes back

The datapath runs in fp16 (inputs converted host-side): full PE rate with
overlapped weight loads; accumulation is always fp32 in PSUM. Total error
vs the f32 reference is ~2e-3.

Each PSUM matmul accumulation group must own a full 2KB bank (a group's
start=True clears the whole bank row); the V projection therefore runs
its N=256 output tiles as N=512 matmuls with the weight chunk duplicated
side by side, discarding the duplicate half of each bank.

The AllToAll runs over all 8 cores (4-core groups are unsupported): shards
are duplicated to both batch groups and the final projection uses 16
virtual hn-chunks whose weights are host-side zero-masked for the chunks
belonging to the other batch. This keeps the graph rank-independent (SPMD).
"""

import numpy as np

import concourse.bass as bass
import concourse.mybir as mybir
import concourse.tile as tile
from concourse import bacc
from concourse.bass_utils import run_bass_kernel_spmd

N_CORES = 8
B = 2
T = 2048
D = 1024
HEADS = 16
HD = 64
HPC = 4            # heads per core
HN = HPC * HD      # 256 head-features per core
TS = T // 4        # 512 time-slice per core after reshard
f32 = mybir.dt.float32
f16 = mybir.dt.float16
EXP = mybir.ActivationFunctionType.Exp

_cached = None


def _build():
    nc = bacc.Bacc("TRN2", target_bir_lowering=False, debug=False,
                   num_devices=N_CORES)

    xqT = nc.dram_tensor("xqT", [D, T], f16, kind="ExternalInput")
    xkT = nc.dram_tensor("xkT", [D, T], f16, kind="ExternalInput")
    xvT = nc.dram_tensor("xvT", [D, T], f16, kind="ExternalInput")
    wqT = nc.dram_tensor("wqT", [D, HN], f16, kind="ExternalInput")
    wkT = nc.dram_tensor("wkT", [D, HN], f16, kind="ExternalInput")
    wvT = nc.dram_tensor("wvT", [D, HN], f16, kind="ExternalInput")
    woT = nc.dram_tensor("woT", [2 * D, D], f16, kind="ExternalInput")
    bo = nc.dram_tensor("bo", [D, 1], f32, kind="ExternalInput")
    out = nc.dram_tensor("out", [D, TS], f32, kind="ExternalOutput")

    onesv_d = nc.inline_tensor(np.ones((128, 64), np.float16), name="onesv_c")

    with tile.TileContext(nc) as tc:
        with (
            tc.tile_pool(name="xp", bufs=3) as xp,
            tc.tile_pool(name="wp", bufs=2) as wp,
            tc.tile_pool(name="ep", bufs=6) as ep,
            tc.tile_pool(name="ps", bufs=4, space="PSUM") as ps,
            tc.tile_pool(name="wop", bufs=3) as wop,
            tc.tile_pool(name="rp", bufs=3) as rp,
            tc.tile_pool(name="yp", bufs=3) as yp,
            tc.tile_pool(name="dram", bufs=1, space="DRAM") as dram,
            tc.tile_pool(name="pers", bufs=1) as pers,
        ):
            # persistent SBUF tensors (one slot per tag)
            QT = [pers.tile([128, T], f16, tag=f"QT{p}", name=f"QT{p}")
                  for p in range(2)]
            KT = [pers.tile([128, T], f16, tag=f"KT{p}", name=f"KT{p}")
                  for p in range(2)]
            V = pers.tile([128, 16, HPC, HD + 1], f16, tag="Vsb", name="Vsb")
            OT = [pers.tile([HD + 1, T], f16, tag=f"OT{h}", name=f"OT{h}")
                  for h in range(HPC)]
            onesf = pers.tile([128, HD], f16, tag="onesf", name="onesf")
            bo_sb = pers.tile([128, 8], f32, tag="bo_sb", name="bo_sb")

            nc.sync.dma_start(onesf[:], onesv_d.ap())
            # ones column of the V operand (softmax denominator)
            nc.sync.dma_start(V[:, :, :, HD:HD + 1], onesv_d.ap())

            # ---- projections ----
            def proj_qk(xdram, wdram, dest):
                wt = wp.tile([128, 8, 512], f16, tag="w", name="wt")
                nc.sync.dma_start(
                    wt[:, :, 0:HN],
                    wdram[:].rearrange("(c p) n -> p c n", p=128),
                )
                psums = [
                    ps.tile([128, 1024], f32, tag="ps", name=f"pp{i}")
                    for i in range(4)
                ]
                for kk in range(8):
                    xt = xp.tile([128, T], f16, tag="x", name="xt")
                    nc.sync.dma_start(xt[:], xdram[kk * 128:(kk + 1) * 128, :])
                    for i in range(4):
                        pair, tq2 = i // 2, i % 2
                        for hf in range(2):
                            nc.tensor.matmul(
                                psums[i][:, hf * 512:(hf + 1) * 512],
                                wt[:, kk, pair * 128:(pair + 1) * 128],
                                xt[:, (tq2 * 2 + hf) * 512:(tq2 * 2 + hf + 1) * 512],
                                start=(kk == 0), stop=(kk == 7),
                            )
                for i in range(4):
                    pair, tq2 = i // 2, i % 2
                    nc.vector.tensor_copy(
                        dest[pair][:, tq2 * 1024:(tq2 + 1) * 1024],
                        psums[i][:],
                    )

            def proj_v(rnd):
                # each accumulation group must own a full PSUM bank: run the
                # N=512 matmul with the weight chunk duplicated side by side
                # (cols 256:512 of each bank are a discarded duplicate)
                wt = wp.tile([128, 8, 512], f16, tag="w", name="wt")
                for half in range(2):
                    nc.sync.dma_start(
                        wt[:, :, half * 256:(half + 1) * 256],
                        wvT[:].rearrange("(c p) n -> p c n", p=128),
                    )
                psums = [
                    ps.tile([128, 1024], f32, tag="ps", name=f"pv{i}")
                    for i in range(4)
                ]
                for kk in range(8):
                    xt = xp.tile([128, T], f16, tag="x", name="xt")
                    nc.sync.dma_start(xt[:], xvT[kk * 128:(kk + 1) * 128, :])
                    for tt8 in range(8):
                        tt = rnd * 8 + tt8
                        nc.tensor.matmul(
                            psums[tt8 // 2][:, (tt8 % 2) * 512:(tt8 % 2 + 1) * 512],
                            xt[:, tt * 128:(tt + 1) * 128],
                            wt[:, kk, :],
                            start=(kk == 0), stop=(kk == 7),
                        )
                for i in range(4):
                    for sub in range(2):
                        tt = rnd * 8 + i * 2 + sub
                        nc.vector.tensor_copy(
                            V[:, tt, :, 0:HD],
                            psums[i][:, sub * 512:sub * 512 + 256]
                            .rearrange("p (h n) -> p h n", h=HPC),
                        )

            proj_v(0)
            proj_v(1)
            proj_qk(xkT, wkT, KT)
            proj_qk(xqT, wqT, QT)

            # bias chunks [128,1] per output d-chunk
            for dd in range(8):
                nc.sync.dma_start(
                    bo_sb[:, dd:dd + 1], bo[dd * 128:(dd + 1) * 128, 0:1]
                )

            # a2a bounce buffers (pair p = local hn rows [128p, 128p+128))
            a2a_in = [
                dram.tile([8, 128, TS], f16, name=f"a2a_in{p}") for p in range(2)
            ]
            a2a_out = [
                dram.tile([8, 128, TS], f16, name=f"a2a_out{p}") for p in range(2)
            ]

            # ---- attention ----
            for h in range(HPC):
                pair, row = h // 2, (h % 2) * HD
                for tq2 in range(2):
                    po = ps.tile([HD + 1, 1024], f32, tag="ps", name="po")
                    for tk in range(16):
                        s2 = ps.tile([128, 1024], f32, tag="ps", name="s2")
                        for hf in range(2):
                            nc.tensor.matmul(
                                s2[:, hf * 512:(hf + 1) * 512],
                                KT[pair][row:row + HD, tk * 128:(tk + 1) * 128],
                                QT[pair][row:row + HD,
                                         (tq2 * 2 + hf) * 512:(tq2 * 2 + hf + 1) * 512],
                                start=True, stop=True,
                            )
                        e = ep.tile([128, 1024], f16, tag="e", name="e")
                        nc.scalar.activation(e[:], s2[:], EXP, scale=0.125)
                        for hf in range(2):
                            nc.tensor.matmul(
                                po[:, hf * 512:(hf + 1) * 512],
                                V[:, tk, h, :],
                                e[:, hf * 512:(hf + 1) * 512],
                                start=(tk == 0), stop=(tk == 15),
                            )
                    nc.vector.tensor_copy(
                        OT[h][:, tq2 * 1024:(tq2 + 1) * 1024], po[:]
                    )
                # normalize by softmax sum (row HD of OT[h]) and stage for a2a
                with nc.allow_low_precision(reason="fp16 datapath by design"):
                    nc.vector.reciprocal(
                        OT[h][HD:HD + 1, :], OT[h][HD:HD + 1, :]
                    )
                for tq in range(4):
                    pb = ps.tile([HD, 512], f32, tag="ps", name="pb")
                    nc.tensor.matmul(
                        pb[:],
                        onesf[HD:HD + 1, :],
                        OT[h][HD:HD + 1, tq * 512:(tq + 1) * 512],
                        start=True, stop=True,
                    )
                    nc.vector.tensor_tensor(
                        OT[h][0:HD, tq * 512:(tq + 1) * 512],
                        OT[h][0:HD, tq * 512:(tq + 1) * 512],
                        pb[:],
                        op=mybir.AluOpType.mult,
                    )
                    for rep in (0, 4):
                        nc.sync.dma_start(
                            a2a_in[pair][tq + rep, row:row + HD, :],
                            OT[h][0:HD, tq * 512:(tq + 1) * 512],
                        )
                if h == 1 or h == 3:
                    p = h // 2
                    nc.gpsimd.collective_compute(
                        "AllToAll",
                        mybir.AluOpType.bypass,
                        replica_groups=[list(range(N_CORES))],
                        ins=[a2a_in[p].opt()],
                        outs=[a2a_out[p].opt()],
                    )

            # ---- output projection over 16 virtual hn chunks ----
            ypsums = [
                ps.tile([128, 1024], f32, tag="ps", name=f"py{i}")
                for i in range(4)
            ]
            cc_list = list(range(0, 16, 2)) + list(range(1, 16, 2))
            for idx, cc in enumerate(cc_list):
                p, blk = cc % 2, cc // 2
                rt = rp.tile([128, TS], f16, tag="r", name="rt")
                nc.sync.dma_start(rt[:], a2a_out[p][blk])
                wt2 = wop.tile([128, 1024], f16, tag="wo", name="wt2")
                nc.sync.dma_start(
                    wt2[:], woT[cc * 128:(cc + 1) * 128, :]
                )
                for dd in range(8):
                    nc.tensor.matmul(
                        ypsums[dd // 2][:, (dd % 2) * 512:(dd % 2 + 1) * 512],
                        wt2[:, dd * 128:(dd + 1) * 128],
                        rt[:],
                        start=(idx == 0), stop=(idx == 15),
                    )
            for dd in range(8):
                y = yp.tile([128, TS], f32, tag="y", name="y")
                nc.vector.tensor_scalar_add(
                    y[:], ypsums[dd // 2][:, (dd % 2) * 512:(dd % 2 + 1) * 512],
                    bo_sb[:, dd:dd + 1],
                )
                nc.sync.dma_start(out[dd * 128:(dd + 1) * 128, :], y[:])

    nc.compile()
    return nc


def _shard_inputs(k, q, v, Wk, Wq, Wv, Wo, bo):
    woT_full = np.ascontiguousarray(Wo.T).astype(np.float16)  # [hn, d]
    in_maps = []
    for c in range(N_CORES):
        i_b, i_h = c // 4, c % 4
        sl = slice(i_h * HN, (i_h + 1) * HN)
        # masked out-projection weights: 16 virtual chunks (cc = 2*blk + p)
        woT_m = np.zeros((2 * D, D), np.float16)
        for cc in range(16):
            p, blk = cc % 2, cc // 2
            if blk // 4 == i_b:
                ghc = 2 * (blk % 4) + p  # global hn chunk 0..7
                woT_m[cc * 128:(cc + 1) * 128, :] = \
                    woT_full[ghc * 128:(ghc + 1) * 128, :]
        in_maps.append({
            "xqT": q[i_b].T.astype(np.float16),
            "xkT": k[i_b].T.astype(np.float16),
            "xvT": v[i_b].T.astype(np.float16),
            "wqT": Wq[sl].T.astype(np.float16),
            "wkT": Wk[sl].T.astype(np.float16),
            "wvT": Wv[sl].T.astype(np.float16),
            "woT": woT_m,
            "bo": np.ascontiguousarray(bo.reshape(D, 1)).astype(np.float32),
        })
    return in_maps


def _run(in_maps, **kw):
    global _cached
    if _cached is None:
        _cached = _build()
    return run_bass_kernel_spmd(_cached, in_maps, core_ids=list(range(N_CORES)),
                                **kw)


def kernel(k, q, v, Wk, Wq, Wv, Wo, bo):
    k, q, v = (np.asarray(x, np.float32) for x in (k, q, v))
    Wk, Wq, Wv, Wo, bo = (np.asarray(x, np.float32) for x in (Wk, Wq, Wv, Wo, bo))
    in_maps = _shard_inputs(k, q, v, Wk, Wq, Wv, Wo, bo)
    res = _run(in_maps)
    out = np.empty((B, T, D), np.float32)
    for c in range(N_CORES):
        i_b, i_h = c // 4, c % 4
        out[i_b, i_h * TS:(i_h + 1) * TS, :] = res.results[c]["out"].T
    return out


# revision 13
# speedup vs baseline: 1.1664x; 1.1664x over previous
"""Distributed multi-head attention for TRN2 (8 NeuronCores).

Problem: b=2, t=2048, d=1024, h=16 heads, head_dim=64.
  out = softmax((q Wq^T)(k Wk^T)^T / 8) (v Wv^T) Wo^T + bo   (per head)

Sharding: core c -> batch i_b = c//4, head group i_h = c%4 (4 heads = 256
features). Each core projects Q/K/V for its batch+heads, runs attention,
then an 8-core AllToAll reshards head-major -> time-major so each core
computes the final projection for its 512-row time slice.

Device layouts (chosen so no on-chip transposes are needed):
  - activations streamed as X^T [d, t]; Q/K kept transposed [hn, t]
  - scores computed directly as S^T [t_k, t_q]; softmax denominator via an
    extra ones-column in the V operand of the P@V matmul
  - output projection computes Y^T [d, t_slice]; host transposes back

The datapath runs in fp16 (inputs converted host-side): full PE rate with
overlapped weight loads; accumulation is always fp32 in PSUM. Total error
vs the f32 reference is ~2e-3.

Each PSUM matmul accumulation group must own a full 2KB bank (a group's
start=True clears the whole bank row); the V projection therefore runs
its N=256 output tiles as N=512 matmuls with the weight chunk duplicated
side by side, discarding the duplicate half of each bank.

The AllToAll runs over all 8 cores (4-core groups are unsupported): shards
are duplicated to both batch groups and the final projection uses 16
virtual hn-chunks whose weights are host-side zero-masked for the chunks
belonging to the other batch. This keeps the graph rank-independent (SPMD).
"""

import numpy as np

import concourse.bass as bass
import concourse.mybir as mybir
import concourse.tile as tile
from concourse import bacc
from concourse.bass_utils import run_bass_kernel_spmd

N_CORES = 8
B = 2
T = 2048
D = 1024
HEADS = 16
HD = 64
HPC = 4            # heads per core
HN = HPC * HD      # 256 head-features per core
TS = T // 4        # 512 time-slice per core after reshard
f32 = mybir.dt.float32
f16 = mybir.dt.float16
EXP = mybir.ActivationFunctionType.Exp

_cached = None


def _build():
    nc = bacc.Bacc("TRN2", target_bir_lowering=False, debug=False,
                   num_devices=N_CORES)

    xqT = nc.dram_tensor("xqT", [D, T], f16, kind="ExternalInput")
    xkT = nc.dram_tensor("xkT", [D, T], f16, kind="ExternalInput")
    xvT = nc.dram_tensor("xvT", [D, T], f16, kind="ExternalInput")
    wqT = nc.dram_tensor("wqT", [D, HN], f16, kind="ExternalInput")
    wkT = nc.dram_tensor("wkT", [D, HN], f16, kind="ExternalInput")
    wvT = nc.dram_tensor("wvT", [D, HN], f16, kind="ExternalInput")
    woT = nc.dram_tensor("woT", [2 * D, D], f16, kind="ExternalInput")
    bo = nc.dram_tensor("bo", [D, 1], f32, kind="ExternalInput")
    out = nc.dram_tensor("out", [D, TS], f32, kind="ExternalOutput")

    onesv_d = nc.inline_tensor(np.ones((128, 64), np.float16), name="onesv_c")

    with tile.TileContext(nc) as tc:
        with (
            tc.tile_pool(name="xp", bufs=10) as xp,
            tc.tile_pool(name="wp", bufs=2) as wp,
            tc.tile_pool(name="ep", bufs=6) as ep,
            tc.tile_pool(name="ps", bufs=4, space="PSUM") as ps,
            tc.tile_pool(name="wop", bufs=3) as wop,
            tc.tile_pool(name="rp", bufs=3) as rp,
            tc.tile_pool(name="yp", bufs=3) as yp,
            tc.tile_pool(name="dram", bufs=1, space="DRAM") as dram,
            tc.tile_pool(name="pers", bufs=1) as pers,
        ):
            # persistent SBUF tensors (one slot per tag)
            QT = [pers.tile([128, T], f16, tag=f"QT{p}", name=f"QT{p}")
                  for p in range(2)]
            KT = [pers.tile([128, T], f16, tag=f"KT{p}", name=f"KT{p}")
                  for p in range(2)]
            V = pers.tile([128, 16, HPC, HD + 1], f16, tag="Vsb", name="Vsb")
            OT = [pers.tile([HD + 1, T], f16, tag=f"OT{h}", name=f"OT{h}")
                  for h in range(HPC)]
            onesf = pers.tile([128, HD], f16, tag="onesf", name="onesf")
            bo_sb = pers.tile([128, 8], f32, tag="bo_sb", name="bo_sb")

            nc.sync.dma_start(onesf[:], onesv_d.ap())
            # ones column of the V operand (softmax denominator)
            nc.sync.dma_start(V[:, :, :, HD:HD + 1], onesv_d.ap())

            # ---- projections ----
            def proj_qk(xdram, wdram, dest):
                wt = wp.tile([128, 8, 512], f16, tag="w", name="wt")
                nc.sync.dma_start(
                    wt[:, :, 0:HN],
                    wdram[:].rearrange("(c p) n -> p c n", p=128),
                )
                psums = [
                    ps.tile([128, 1024], f32, tag="ps", name=f"pp{i}")
                    for i in range(4)
                ]
                for kk in range(8):
                    xt = xp.tile([128, T], f16, tag="x", name="xt")
                    nc.sync.dma_start(xt[:], xdram[kk * 128:(kk + 1) * 128, :])
                    for i in range(4):
                        pair, tq2 = i // 2, i % 2
                        for hf in range(2):
                            nc.tensor.matmul(
                                psums[i][:, hf * 512:(hf + 1) * 512],
                                wt[:, kk, pair * 128:(pair + 1) * 128],
                                xt[:, (tq2 * 2 + hf) * 512:(tq2 * 2 + hf + 1) * 512],
                                start=(kk == 0), stop=(kk == 7),
                            )
                for i in range(4):
                    pair, tq2 = i // 2, i % 2
                    nc.vector.tensor_copy(
                        dest[pair][:, tq2 * 1024:(tq2 + 1) * 1024],
                        psums[i][:],
                    )

            def proj_v():
                # each accumulation group must own a full PSUM bank: the two
                # N=256 groups of a [128,1024] slot sit at cols 0:256 and
                # 512:768 (one per bank). 8 banks cover 8 t-tiles, so the 16
                # t-tiles run as two matmul rounds over the same X chunks
                # (kept resident in SBUF).
                wt = wp.tile([128, 8, 512], f16, tag="w", name="wt")
                nc.sync.dma_start(
                    wt[:, :, 0:HN],
                    wvT[:].rearrange("(c p) n -> p c n", p=128),
                )
                xts = []
                for kk in range(8):
                    xt = xp.tile([128, T], f16, tag="x", name="xt")
                    nc.sync.dma_start(xt[:], xvT[kk * 128:(kk + 1) * 128, :])
                    xts.append(xt)
                for rnd in range(2):
                    psums = [
                        ps.tile([128, 1024], f32, tag="ps", name=f"pv{i}")
                        for i in range(4)
                    ]
                    for kk in range(8):
                        for tt8 in range(8):
                            tt = rnd * 8 + tt8
                            nc.tensor.matmul(
                                psums[tt8 // 2][:, (tt8 % 2) * 512:
                                                (tt8 % 2) * 512 + 256],
                                xts[kk][:, tt * 128:(tt + 1) * 128],
                                wt[:, kk, 0:HN],
                                start=(kk == 0), stop=(kk == 7),
                            )
                    for i in range(4):
                        for sub in range(2):
                            tt = rnd * 8 + i * 2 + sub
                            nc.vector.tensor_copy(
                                V[:, tt, :, 0:HD],
                                psums[i][:, sub * 512:sub * 512 + 256]
                                .rearrange("p (h n) -> p h n", h=HPC),
                            )

            proj_v()
            proj_qk(xkT, wkT, KT)
            proj_qk(xqT, wqT, QT)

            # bias chunks [128,1] per output d-chunk
            for dd in range(8):
                nc.sync.dma_start(
                    bo_sb[:, dd:dd + 1], bo[dd * 128:(dd + 1) * 128, 0:1]
                )

            # a2a bounce buffers (pair p = local hn rows [128p, 128p+128))
            a2a_in = [
                dram.tile([8, 128, TS], f16, name=f"a2a_in{p}") for p in range(2)
            ]
            a2a_out = [
                dram.tile([8, 128, TS], f16, name=f"a2a_out{p}") for p in range(2)
            ]

            # ---- attention ----
            # normalize by softmax sum (row HD of OT[h]) and stage for a2a.
            # Emission is deferred into the NEXT (h, tq2) block so the PE's
            # broadcast matmuls queue behind ready score work instead of
            # stalling in-order on the DVE reciprocal chain.
            def normalize(h, tq2):
                pair, row = h // 2, (h % 2) * HD
                with nc.allow_low_precision(reason="fp16 datapath by design"):
                    nc.vector.reciprocal(
                        OT[h][HD:HD + 1, tq2 * 1024:(tq2 + 1) * 1024],
                        OT[h][HD:HD + 1, tq2 * 1024:(tq2 + 1) * 1024],
                    )
                for tq in (2 * tq2, 2 * tq2 + 1):
                    pb = ps.tile([HD, 512], f32, tag="ps", name="pb")
                    nc.tensor.matmul(
                        pb[:],
                        onesf[HD:HD + 1, :],
                        OT[h][HD:HD + 1, tq * 512:(tq + 1) * 512],
                        start=True, stop=True,
                    )
                    nc.vector.tensor_tensor(
                        OT[h][0:HD, tq * 512:(tq + 1) * 512],
                        OT[h][0:HD, tq * 512:(tq + 1) * 512],
                        pb[:],
                        op=mybir.AluOpType.mult,
                    )
                    for rep in (0, 4):
                        nc.sync.dma_start(
                            a2a_in[pair][tq + rep, row:row + HD, :],
                            OT[h][0:HD, tq * 512:(tq + 1) * 512],
                        )

            def a2a(p):
                nc.gpsimd.collective_compute(
                    "AllToAll",
                    mybir.AluOpType.bypass,
                    replica_groups=[list(range(N_CORES))],
                    ins=[a2a_in[p].opt()],
                    outs=[a2a_out[p].opt()],
                )

            # Both tq2 streams of a head run interleaved, with the P@V
            # matmuls lagging the score matmuls by one tk step so the PE's
            # in-order stream never waits on the exp of the current step.
            # Deferred work (previous head's normalize + collective trigger)
            # is injected mid-loop.
            def emit_head(h, defer):
                pair, row = h // 2, (h % 2) * HD
                po = [ps.tile([HD + 1, 1024], f32, tag="ps", name=f"po{s}")
                      for s in range(2)]
                es = {}
                for tk in range(16):
                    for tq2 in range(2):
                        s2 = ps.tile([128, 1024], f32, tag="ps", name="s2")
                        for hf in range(2):
                            nc.tensor.matmul(
                                s2[:, hf * 512:(hf + 1) * 512],
                                KT[pair][row:row + HD, tk * 128:(tk + 1) * 128],
                                QT[pair][row:row + HD,
                                         (tq2 * 2 + hf) * 512:(tq2 * 2 + hf + 1) * 512],
                                start=True, stop=True,
                            )
                        e = ep.tile([128, 1024], f16, tag="e", name="e")
                        nc.scalar.activation(e[:], s2[:], EXP, scale=0.125)
                        es[tq2] = e
                    if tk > 0:
                        for tq2 in range(2):
                            for hf in range(2):
                                nc.tensor.matmul(
                                    po[tq2][:, hf * 512:(hf + 1) * 512],
                                    V[:, tk - 1, h, :],
                                    prev_es[tq2][:, hf * 512:(hf + 1) * 512],
                                    start=(tk == 1), stop=False,
                                )
                    prev_es = dict(es)
                    if tk == 5 and defer:
                        defer.pop(0)()
                    if tk == 10 and defer:
                        defer.pop(0)()
                for tq2 in range(2):
                    for hf in range(2):
                        nc.tensor.matmul(
                            po[tq2][:, hf * 512:(hf + 1) * 512],
                            V[:, 15, h, :],
                            prev_es[tq2][:, hf * 512:(hf + 1) * 512],
                            start=False, stop=True,
                        )
                while defer:
                    defer.pop(0)()
                for tq2 in range(2):
                    nc.vector.tensor_copy(
                        OT[h][:, tq2 * 1024:(tq2 + 1) * 1024], po[tq2]
                    )

            emit_head(0, [])
            emit_head(1, [lambda: normalize(0, 0), lambda: normalize(0, 1)])
            emit_head(2, [lambda: normalize(1, 0),
                          lambda: (normalize(1, 1), a2a(0))])
            emit_head(3, [lambda: normalize(2, 0), lambda: normalize(2, 1)])
            normalize(3, 0)
            normalize(3, 1)
            a2a(1)

            # ---- output projection over 16 virtual hn chunks ----
            ypsums = [
                ps.tile([128, 1024], f32, tag="ps", name=f"py{i}")
                for i in range(4)
            ]
            cc_list = list(range(0, 16, 2)) + list(range(1, 16, 2))
            for idx, cc in enumerate(cc_list):
                p, blk = cc % 2, cc // 2
                rt = rp.tile([128, TS], f16, tag="r", name="rt")
                nc.sync.dma_start(rt[:], a2a_out[p][blk])
                wt2 = wop.tile([128, 1024], f16, tag="wo", name="wt2")
                nc.sync.dma_start(
                    wt2[:], woT[cc * 128:(cc + 1) * 128, :]
                )
                for dd in range(8):
                    nc.tensor.matmul(
                        ypsums[dd // 2][:, (dd % 2) * 512:(dd % 2 + 1) * 512],
                        wt2[:, dd * 128:(dd + 1) * 128],
                        rt[:],
                        start=(idx == 0), stop=(idx == 15),
                    )
            for dd in range(8):
                y = yp.tile([128, TS], f32, tag="y", name="y")
                nc.vector.tensor_scalar_add(
                    y[:], ypsums[dd // 2][:, (dd % 2) * 512:(dd % 2 + 1) * 512],
                    bo_sb[:, dd:dd + 1],
                )
                nc.sync.dma_start(out[dd * 128:(dd + 1) * 128, :], y[:])

    nc.compile()
    return nc


def _shard_inputs(k, q, v, Wk, Wq, Wv, Wo, bo):
    woT_full = np.ascontiguousarray(Wo.T).astype(np.float16)  # [hn, d]
    in_maps = []
    for c in range(N_CORES):
        i_b, i_h = c // 4, c % 4
        sl = slice(i_h * HN, (i_h + 1) * HN)
        # masked out-projection weights: 16 virtual chunks (cc = 2*blk + p)
        woT_m = np.zeros((2 * D, D), np.float16)
        for cc in range(16):
            p, blk = cc % 2, cc // 2
            if blk // 4 == i_b:
                ghc = 2 * (blk % 4) + p  # global hn chunk 0..7
                woT_m[cc * 128:(cc + 1) * 128, :] = \
                    woT_full[ghc * 128:(ghc + 1) * 128, :]
        in_maps.append({
            "xqT": q[i_b].T.astype(np.float16),
            "xkT": k[i_b].T.astype(np.float16),
            "xvT": v[i_b].T.astype(np.float16),
            "wqT": Wq[sl].T.astype(np.float16),
            "wkT": Wk[sl].T.astype(np.float16),
            "wvT": Wv[sl].T.astype(np.float16),
            "woT": woT_m,
            "bo": np.ascontiguousarray(bo.reshape(D, 1)).astype(np.float32),
        })
    return in_maps


def _run(in_maps, **kw):
    global _cached
    if _cached is None:
        _cached = _build()
    return run_bass_kernel_spmd(_cached, in_maps, core_ids=list(range(N_CORES)),
                                **kw)


def kernel(k, q, v, Wk, Wq, Wv, Wo, bo):
    k, q, v = (np.asarray(x, np.float32) for x in (k, q, v))
    Wk, Wq, Wv, Wo, bo = (np.asarray(x, np.float32) for x in (Wk, Wq, Wv, Wo, bo))
    in_maps = _shard_inputs(k, q, v, Wk, Wq, Wv, Wo, bo)
    res = _run(in_maps)
    out = np.empty((B, T, D), np.float32)
    for c in range(N_CORES):
        i_b, i_h = c // 4, c % 4
        out[i_b, i_h * TS:(i_h + 1) * TS, :] = res.results[c]["out"].T
    return out
